# revision 1
# baseline (speedup 1.0000x reference)
"""Trainium2 Bass kernel: per-tensor asymmetric int8 activation quantization
followed by a linear layer (y = quantize(x) @ W.T + bias).

Sharding (8 cores): 4-way over tokens x 2-way over out_features.
Each core receives:
  xT   [D_IN, TOK_C]   fp32  (x transposed, token-sharded)
  wT   [D_IN, DOUT_C]  fp16  (W transposed, out_feature-sharded)
  bias [DOUT_C]        fp16
and produces y [TOK_C, DOUT_C] fp32.

Device program per core:
  phase 0: streaming min/max over the local x shard (DVE reduce + GPSIMD
           partition all-reduce), then an 8-core AllReduce(max) of
           [xmax, -xmin] to get the global per-tensor range.
  scalars: inv_scale = 255/(xmax-xmin); zp = clip(-128 - rne(xmin/scale));
           rne() implemented with the fp32 magic constant 1.5*2^23.
  main:    for each 128-token block: quantize (ACT fused scale+magic, DVE
           zero-point + clip, cast fp16 -- q is integer in [-128,127] so
           fp16 is exact), then fp16 matmuls accumulate fp32 into PSUM with
           the weight tensor resident in SBUF; bias is folded in as a K=1
           matmul against a ones vector; result DMAd out in natural
           [token, dout] layout.
"""

import sys

import numpy as np

try:  # the grading environment may or may not have concourse on sys.path
    import concourse  # noqa: F401
except ImportError:  # pragma: no cover
    sys.path.insert(0, "/opt/trn_rl_repo")

P = 128
MAGIC = 12582912.0  # 1.5 * 2**23: fp32 add/sub rounds to nearest-even integer
QMIN, QMAX = -128.0, 127.0
PH0_TTR = False  # fused tensor_tensor_reduce in phase 0 (hangs TRN2 HW; keep off)

# Full-problem shape (hardcoded per contract; kernel() checks them)
B, S, D_IN, D_OUT = 4, 2048, 4096, 4096
R_SHARDS, G_SHARDS = 4, 2  # token shards x out_feature shards
N_CORES = 8


def build_program(d_in, tok, dout, n_cores=N_CORES, w_passes=1, bias_mode="matmul"):
    """Emit the per-core SPMD program. Returns a compiled Bacc object.

    w_passes=2 adds a second accumulation pass against a residual weight
    input ("wLo") for near-fp32 weight precision at 2x PE cost.
    bias_mode: "matmul" folds bias in as a K=1 matmul; "evict" adds it
    during PSUM eviction on the vector engine (no K=1 weight loads).
    """
    from contextlib import ExitStack

    import concourse.bacc as bacc
    import concourse.tile as tile
    from concourse import bass_isa, mybir

    f32, f16 = mybir.dt.float32, mybir.dt.float16
    AF = mybir.ActivationFunctionType
    ALU = mybir.AluOpType
    AX = mybir.AxisListType

    assert d_in % P == 0 and tok % P == 0
    assert tok <= dout  # phase-0 reuses the [P, dout] output-pool slots
    KB, MB = d_in // P, tok // P
    KB0 = KB // 2  # phase-0 min/max half (the other half is on the g-sibling core)
    NMM = min(512, dout)
    assert dout % NMM == 0
    NB = dout // NMM

    nc = bacc.Bacc(
        "TRN2",
        target_bir_lowering=False,
        debug=False,
        num_devices=n_cores,
        enable_asserts=False,
    )

    xT = nc.dram_tensor("xT", [d_in, tok], f32, kind="ExternalInput").ap()
    wT = nc.dram_tensor("wT", [d_in, dout], f16, kind="ExternalInput").ap()
    bias = nc.dram_tensor("bias", [dout], f16, kind="ExternalInput").ap()
    w_ins = [wT]
    if w_passes == 2:
        w_ins.append(nc.dram_tensor("wLo", [d_in, dout], f16, kind="ExternalInput").ap())
    y = nc.dram_tensor("y", [tok, dout], f32, kind="ExternalOutput").ap()
    cc_in = nc.dram_tensor("cc_in", [2], f32).ap()
    cc_out = nc.dram_tensor("cc_out", [2], f32, addr_space="Shared").ap()

    x_view = xT.rearrange("(kb p) t -> p kb t", p=P)  # [P, KB, tok]
    w_views = [w.rearrange("(kb p) o -> p kb o", p=P) for w in w_ins]

    with tile.TileContext(nc) as tc, ExitStack() as ctx:
        wpool = ctx.enter_context(tc.tile_pool(name="w", bufs=1))
        xpool = ctx.enter_context(tc.tile_pool(name="x", bufs=2))
        qpool = ctx.enter_context(tc.tile_pool(name="q", bufs=2))
        opool = ctx.enter_context(tc.tile_pool(name="o", bufs=3))
        spool = ctx.enter_context(tc.tile_pool(name="s", bufs=1))
        ppool = ctx.enter_context(tc.tile_pool(name="ps", bufs=2, space="PSUM"))

        # ---- phase 0: min/max over the first half of this core's x shard
        # (the g-sibling core covers the other half; the host rolls the d_in
        # axis for g=1 cores so "first half" differs between siblings).
        # Full-row tiles: 8KB contiguous per partition -> full DMA rate.
        smax = spool.tile([P, KB0], f32)
        smin = spool.tile([P, KB0], f32)
        ph0_dmas = []
        FMAX = 3.0e38
        half = tok // 2
        for kb in range(KB0):
            # alternate pools for ~5 effective prefetch slots at no SBUF cost
            # (the q slots are [P, KB*P] fp16 = the same bytes as [P, tok] f32)
            if kb % 2 == 0:
                x_p = opool.tile([P, dout], f32, tag="o_m")
            else:
                x_p = qpool.tile([P, tok], f32, tag="q_m")
            x_row = x_p[:, 0:tok]
            # alternate DMA queues (the Scalar queue is idle until W loads)
            eng = nc.sync if kb % 2 == 0 else nc.scalar
            ph0_dmas.append(eng.dma_start(x_row, x_view[:, kb, :]))
            if PH0_TTR:
                # fused pairwise-op + reduce: one pass consumes two elems/cycle
                scr = ppool.tile([P, half], f32, tag="psum")
                nc.vector.tensor_tensor_reduce(
                    out=scr[:],
                    in0=x_row[:, 0:half],
                    in1=x_row[:, half:tok],
                    scale=1.0,
                    scalar=-FMAX,
                    op0=ALU.max,
                    op1=ALU.max,
                    accum_out=smax[:, kb : kb + 1],
                )
                scr2 = ppool.tile([P, half], f32, tag="psum")
                nc.vector.tensor_tensor_reduce(
                    out=scr2[:],
                    in0=x_row[:, 0:half],
                    in1=x_row[:, half:tok],
                    scale=1.0,
                    scalar=FMAX,
                    op0=ALU.min,
                    op1=ALU.min,
                    accum_out=smin[:, kb : kb + 1],
                )
            else:
                nc.vector.tensor_reduce(
                    smax[:, kb : kb + 1], x_row, axis=AX.X, op=ALU.max
                )
                nc.vector.tensor_reduce(
                    smin[:, kb : kb + 1], x_row, axis=AX.X, op=ALU.min
                )

        # Resident weights on the Scalar engine's HWDGE queue, staggered
        # behind phase-0's x traffic so they don't compete for HBM fabric.
        w_sbs = []
        n_chunks = 4
        step = max(1, KB // n_chunks)
        chunk_no = 0
        for wi, wv in enumerate(w_views):
            w_sb = wpool.tile([P, KB, dout], f16, tag=f"wsb{wi}")
            for k0 in range(0, KB, step):
                k1 = min(KB, k0 + step)
                wdma = nc.scalar.dma_start(w_sb[:, k0:k1, :], wv[:, k0:k1, :])
                tile.add_dep_helper(
                    wdma.ins, ph0_dmas[-1].ins, reason="W loads after phase-0 x traffic"
                )
                chunk_no += 1
            w_sbs.append(w_sb)
        if bias_mode == "matmul":
            bias_row = wpool.tile([1, dout], f16)
            nc.scalar.dma_start(bias_row[:], bias[None, :])
            ones_t = wpool.tile([1, P], f16)
            nc.vector.memset(ones_t[:], 1.0)
        else:
            bias_bc = wpool.tile([P, dout], f16)
            nc.scalar.dma_start(bias_bc[0:1, :], bias[None, :])
            nc.gpsimd.partition_broadcast(bias_bc[:], bias_bc[0:1, :], channels=P)

        pk = spool.tile([P, 2], f32)
        nc.vector.tensor_reduce(pk[:, 0:1], smax[:], axis=AX.X, op=ALU.max)
        nc.vector.tensor_reduce(pk[:, 1:2], smin[:], axis=AX.X, op=ALU.min)
        nc.vector.tensor_scalar_mul(pk[:, 1:2], pk[:, 1:2], -1.0)
        pkr = spool.tile([P, 2], f32)
        nc.gpsimd.partition_all_reduce(
            pkr[:], pk[:], channels=P, reduce_op=bass_isa.ReduceOp.max
        )

        # ---- 8-core AllReduce(max) of [xmax, -xmin] ----
        sc = spool.tile([1, 2], f32)
        sem_in = nc.alloc_semaphore("ar_in")
        sem_cc = nc.alloc_semaphore("ar_cc")
        sem_out = nc.alloc_semaphore("ar_out")
        with tc.tile_critical():
            nc.gpsimd.dma_start(cc_in[None, :], pkr[0:1, :]).then_inc(sem_in, 16)
            nc.gpsimd.wait_ge(sem_in, 16)
            nc.gpsimd.collective_compute(
                "AllReduce",
                ALU.max,
                replica_groups=[list(range(n_cores))],
                ins=[cc_in],
                outs=[cc_out],
            ).then_inc(sem_cc, 1)
            nc.gpsimd.wait_ge(sem_cc, 1)
            nc.gpsimd.dma_start(sc[:], cc_out[None, :]).then_inc(sem_out, 16)
            nc.gpsimd.wait_ge(sem_out, 16)

        # ---- scalar math: inv_scale, zp ----
        scr = spool.tile([1, 6], f32)
        rng, inv, isc, nt, zp, mzp = (scr[0:1, i : i + 1] for i in range(6))
        nc.vector.tensor_add(rng, sc[0:1, 0:1], sc[0:1, 1:2])  # xmax - xmin
        nc.vector.reciprocal(inv, rng)
        nc.vector.tensor_scalar_mul(isc, inv, 255.0)  # 255/(xmax-xmin) ~ 1/scale
        nc.vector.tensor_mul(nt, sc[0:1, 1:2], isc)  # (-xmin)/scale
        # rne(nt); then zp = clip(-128 + rne(nt), -128, 127)
        nc.vector.tensor_scalar(zp, nt, MAGIC, -MAGIC, op0=ALU.add, op1=ALU.add)
        nc.vector.tensor_scalar(zp, zp, -128.0, -128.0, op0=ALU.add, op1=ALU.max)
        nc.vector.tensor_scalar_min(zp, zp, 127.0)
        nc.vector.tensor_scalar(mzp, zp, -1.0, MAGIC, op0=ALU.mult, op1=ALU.add)
        bc0 = spool.tile([1, 2], f32)
        nc.vector.tensor_copy(bc0[0:1, 0:1], isc)
        nc.vector.tensor_copy(bc0[0:1, 1:2], mzp)
        bc = spool.tile([P, 2], f32)
        nc.gpsimd.partition_broadcast(bc[:], bc0[:], channels=P)

        # ---- main loop: quantize + matmul per 128-token block ----
        for mb in range(MB):
            x_m = xpool.tile([P, KB * P], f32, tag="xm")
            x_m3 = x_m.rearrange("p (a b) -> p a b", b=P)  # [P, KB, P]
            nc.sync.dma_start(x_m3, x_view[:, :, mb * P : (mb + 1) * P])
            # v = x * inv_scale + MAGIC  (ACT); upper bits now hold rne(x/scale)
            nc.scalar.activation(x_m[:], x_m[:], AF.Copy, bias=MAGIC, scale=bc[:, 0:1])
            # v - (MAGIC - zp) = rne(x/scale) + zp ; clamp low
            nc.vector.tensor_scalar(
                x_m[:], x_m[:], bc[:, 1:2], QMIN, op0=ALU.subtract, op1=ALU.max
            )
            q_m = qpool.tile([P, KB, P], f16)
            nc.vector.tensor_scalar(q_m[:], x_m3, QMAX, None, op0=ALU.min)

            psum = ppool.tile([P, dout], f32)
            if bias_mode == "matmul":
                for n in range(NB):
                    nc.tensor.matmul(
                        psum[:, n * NMM : (n + 1) * NMM],
                        ones_t[:],
                        bias_row[:, n * NMM : (n + 1) * NMM],
                        start=True,
                        stop=False,
                    )
            last_wi = len(w_sbs) - 1
            for wi, w_sb in enumerate(w_sbs):
                for kb in range(KB):
                    lhsT = q_m[:, kb, :]
                    for n in range(NB):
                        nc.tensor.matmul(
                            psum[:, n * NMM : (n + 1) * NMM],
                            lhsT,
                            w_sb[:, kb, n * NMM : (n + 1) * NMM],
                            start=(bias_mode != "matmul" and wi == 0 and kb == 0),
                            stop=(kb == KB - 1 and wi == last_wi),
                        )
            o_m = opool.tile([P, dout], f32, tag="o_m")
            if bias_mode == "matmul":
                nc.scalar.copy(o_m[:], psum[:])
            else:
                nc.vector.scalar_tensor_tensor(
                    o_m[:], psum[:], 1.0, bias_bc[:], op0=ALU.mult, op1=ALU.add
                )
            nc.gpsimd.dma_start(y[mb * P : (mb + 1) * P, :], o_m[:])

    nc.compile()
    _dedupe_ldweights(nc)
    return nc


def _dedupe_ldweights(nc):
    """Remove back-to-back InstLdweights with identical weight access patterns.

    bacc's matmul split emits one Ldweights per Matmult even when consecutive
    matmuls share the stationary operand (our 4 n-slices per k-block). The PE
    keeps the stationary operand loaded between matmuls, so a repeat load with
    the same AP is pure overhead (~108ns each, ~half exposed). Only drop
    loads that carry no semaphore waits/updates.
    """
    from concourse import mybir

    for fn in nc.m.functions:
        for bb in fn.blocks:
            insts = bb.instructions
            keep = []
            last_ldw_key = None
            removed = 0
            for inst in insts:
                tname = type(inst).__name__
                if tname == "InstLdweights":
                    key = inst.concise()
                    if (
                        key == last_ldw_key
                        and not inst.has_wait()
                        and not inst.has_update()
                    ):
                        removed += 1
                        continue
                    last_ldw_key = key
                elif tname == "InstMatmult":
                    pass  # matmuls stream; they don't disturb loaded weights
                elif getattr(inst, "engine", None) == mybir.EngineType.PE and tname not in (
                    "InstEventSemaphore",
                    "InstNop",
                ):
                    # any other PE instruction: be conservative
                    last_ldw_key = None
                keep.append(inst)
            if removed:
                del insts[:]
                for inst in keep:
                    insts.append(inst)


def make_in_maps(
    x, weight, bias, r_shards=R_SHARDS, g_shards=G_SHARDS, w_passes=1, bias_mode="matmul"
):
    """Host-side shard/layout prep. Returns (in_maps, tok_c, dout_c)."""
    x = np.asarray(x, dtype=np.float32)
    weight = np.asarray(weight, dtype=np.float32)
    bias = np.asarray(bias, dtype=np.float32)
    tok_tot = int(np.prod(x.shape[:-1]))
    d_in = x.shape[-1]
    d_out = weight.shape[0]
    tok_c = tok_tot // r_shards
    dout_c = d_out // g_shards

    xt = np.ascontiguousarray(x.reshape(tok_tot, d_in).T)  # [d_in, tok_tot]
    b16 = bias.astype(np.float16)
    # g=1 cores get the d_in axis rolled by half so the SPMD program's
    # phase-0 min/max pass (which always scans the first d_in/2 rows) covers
    # the other half of x on the sibling core. Contraction order is
    # irrelevant to the matmul as long as xT and wT are rolled identically.
    half = d_in // 2

    def _roll(a, g):
        return a if g % 2 == 0 else np.concatenate([a[half:], a[:half]], axis=0)

    w_hi, w_lo = [], []
    for g in range(g_shards):
        wg = weight[g * dout_c : (g + 1) * dout_c, :].T  # [d_in, dout_c] fp32
        wg = _roll(wg, g)
        hi = wg.astype(np.float16)
        w_hi.append(np.ascontiguousarray(hi))
        if w_passes == 2:
            w_lo.append(np.ascontiguousarray((wg - hi.astype(np.float32)).astype(np.float16)))

    in_maps = []
    for c in range(r_shards * g_shards):
        r, g = divmod(c, g_shards)
        m = {
            "xT": np.ascontiguousarray(_roll(xt[:, r * tok_c : (r + 1) * tok_c], g)),
            "wT": w_hi[g],
            "bias": np.ascontiguousarray(b16[g * dout_c : (g + 1) * dout_c]),
        }
        if w_passes == 2:
            m["wLo"] = w_lo[g]
        in_maps.append(m)
    return in_maps, tok_c, dout_c


def assemble_output(results, out_shape, tok_c, dout_c, g_shards=G_SHARDS):
    d_out = out_shape[-1]
    tok_tot = int(np.prod(out_shape[:-1]))
    Y = np.empty((tok_tot, d_out), np.float32)
    for c, res in enumerate(results):
        r, g = divmod(c, g_shards)
        Y[r * tok_c : (r + 1) * tok_c, g * dout_c : (g + 1) * dout_c] = res["y"]
    return Y.reshape(out_shape)


_PROGRAM_CACHE = {}


def _get_program(d_in, tok_c, dout_c, w_passes, bias_mode):
    key = (d_in, tok_c, dout_c, w_passes, bias_mode)
    if key not in _PROGRAM_CACHE:
        _PROGRAM_CACHE[key] = build_program(
            d_in, tok_c, dout_c, N_CORES, w_passes, bias_mode
        )
    return _PROGRAM_CACHE[key]


def kernel(x, weight, bias, w_passes=1, bias_mode="matmul", trace=False):
    """Full-input entry point: shards across 8 NeuronCores, runs, gathers."""
    from concourse.bass_utils import run_bass_kernel_spmd

    assert x.shape == (B, S, D_IN) and weight.shape == (D_OUT, D_IN)
    in_maps, tok_c, dout_c = make_in_maps(
        x, weight, bias, w_passes=w_passes, bias_mode=bias_mode
    )
    nc = _get_program(D_IN, tok_c, dout_c, w_passes, bias_mode)
    out = run_bass_kernel_spmd(nc, in_maps, list(range(N_CORES)), trace=trace)
    res = assemble_output(out.results, (B, S, D_OUT), tok_c, dout_c)
    if trace:
        return res, out
    return res



# revision 5
# speedup vs baseline: 1.0416x; 1.0416x over previous
"""Trainium2 Bass kernel: per-tensor asymmetric int8 activation quantization
followed by a linear layer (y = quantize(x) @ W.T + bias).

Sharding (8 cores): 4-way over tokens x 2-way over out_features.
Each core receives:
  xB   [MB, P, KB, P]  fp32  (x blocked per 128-token tile; per-core tile order)
  wT   [D_IN, DOUT_C]  fp16  (W transposed, out_feature-sharded)
  bias [DOUT_C]        fp16
and produces y [TOK_C, DOUT_C] fp32 (rows in the core's tile order; host unpermutes).

Device program per core:
  phase 0: stream the first 8 x tiles (= half this core's tokens; the g-sibling
           core scans the other half via a host-side tile permutation) and
           min/max-reduce them on DVE. Tiles 0-1 stay resident in SBUF for the
           main loop; tiles 2-7 go through scratch. Then GPSIMD partition
           all-reduce + an 8-core AllReduce(max) of [xmax, -xmin].
  weights: 8 chunk DMAs gated on the *completion* of the phase-0 scan (dep on
           the last DVE reduce), so they never steal HBM bandwidth from the
           latency-critical scan.
  scalars: inv_scale = 255/(xmax-xmin); zp = clip(-128 - rne(xmin/scale));
           rne() via the fp32 magic constant 1.5*2^23.
  main:    per 128-token tile: quantize (ACT fused scale+magic, DVE zero-point
           + clip, cast fp16 -- q is integer in [-128,127] so fp16 is exact),
           fp16 matmuls accumulate fp32 into PSUM against SBUF-resident
           weights; bias is added during PSUM eviction on DVE (no K=1 bias
           matmuls); y DMAd out per 512-col slice.
"""

import sys

import numpy as np

try:  # the grading environment may or may not have concourse on sys.path
    import concourse  # noqa: F401
except ImportError:  # pragma: no cover
    sys.path.insert(0, "/opt/trn_rl_repo")

P = 128
MAGIC = 12582912.0  # 1.5 * 2**23: fp32 add/sub rounds to nearest-even integer
QMIN, QMAX = -128.0, 127.0

# Full-problem shape (hardcoded per contract; kernel() checks them)
B, S, D_IN, D_OUT = 4, 2048, 4096, 4096
R_SHARDS, G_SHARDS = 4, 2  # token shards x out_feature shards
N_CORES = 8


def build_program(d_in, tok, dout, n_cores=N_CORES):
    """Emit the per-core SPMD program. Returns a compiled Bacc object."""
    from contextlib import ExitStack

    import concourse.bacc as bacc
    import concourse.tile as tile
    from concourse import bass_isa, mybir

    f32, f16 = mybir.dt.float32, mybir.dt.float16
    AF = mybir.ActivationFunctionType
    ALU = mybir.AluOpType
    AX = mybir.AxisListType

    assert d_in % P == 0 and tok % P == 0
    KB, MB = d_in // P, tok // P
    SCAN_MB = MB // 2  # tiles scanned for min/max (g-sibling scans the rest)
    NMM = min(512, dout)
    assert dout % NMM == 0
    NB = dout // NMM
    KHALF = KB // 2
    W_CHUNKS = 8
    KW = KB // W_CHUNKS

    nc = bacc.Bacc(
        "TRN2",
        target_bir_lowering=False,
        debug=False,
        num_devices=n_cores,
        enable_asserts=False,
    )

    xB = nc.dram_tensor("xB", [MB, P, KB, P], f32, kind="ExternalInput").ap()
    wT = nc.dram_tensor("wT", [d_in, dout], f16, kind="ExternalInput").ap()
    bias = nc.dram_tensor("bias", [dout], f16, kind="ExternalInput").ap()
    y = nc.dram_tensor("y", [tok, dout], f32, kind="ExternalOutput").ap()
    cc_in = nc.dram_tensor("cc_in", [2], f32).ap()
    cc_out = nc.dram_tensor("cc_out", [2], f32, addr_space="Shared").ap()

    w_view = wT.rearrange("(kb p) o -> p kb o", p=P)  # [P, KB, dout]

    with tile.TileContext(nc) as tc, ExitStack() as ctx:
        wpool = ctx.enter_context(tc.tile_pool(name="w", bufs=1))
        xpool = ctx.enter_context(tc.tile_pool(name="x", bufs=2))
        qpool = ctx.enter_context(tc.tile_pool(name="q", bufs=2))
        opool = ctx.enter_context(tc.tile_pool(name="o", bufs=3))
        spool = ctx.enter_context(tc.tile_pool(name="s", bufs=1))
        ppool = ctx.enter_context(tc.tile_pool(name="ps", bufs=2, space="PSUM"))

        # ---- phase 0: min/max over the first SCAN_MB x tiles (full d_in).
        # Tiles 0-1 land in their main-loop xpool slots and stay resident;
        # tiles 2..7 stream through scratch slots as two [P, KB/2*P] halves
        # (the o/q pool slots are 8KB/partition; a full tile is 16KB).
        n_scan_cols = 2 + (SCAN_MB - 2) * 2
        smax = spool.tile([P, n_scan_cols], f32)
        smin = spool.tile([P, n_scan_cols], f32)
        x_resident = []
        last_reduce = None
        col = 0
        dma_i = 0
        for i in range(SCAN_MB):
            if i < 2:
                x_m = xpool.tile([P, KB * P], f32, tag="xm")
                eng = nc.sync if dma_i % 2 == 0 else nc.scalar
                dma_i += 1
                eng.dma_start(
                    x_m.rearrange("p (a b) -> p a b", b=P), xB[i, :, :, :]
                )
                x_resident.append(x_m)
                r1 = nc.vector.tensor_reduce(
                    smax[:, col : col + 1], x_m[:], axis=AX.X, op=ALU.max
                )
                r2 = nc.vector.tensor_reduce(
                    smin[:, col : col + 1], x_m[:], axis=AX.X, op=ALU.min
                )
                last_reduce = r2
                col += 1
            else:
                for h in range(2):
                    # scratch: o/q pool slots are 8KB/partition; an f32 tile of
                    # [P, KHALF*P] is the same byte size as their main-loop use
                    if (i * 2 + h) % 2 == 0:
                        x_p = opool.tile([P, KHALF * P], f32, tag="o_m")
                    else:
                        x_p = qpool.tile([P, KHALF * P], f32, tag="q_m")
                    x_row = x_p[:]
                    eng = nc.sync if dma_i % 2 == 0 else nc.scalar
                    dma_i += 1
                    eng.dma_start(
                        x_row.rearrange("p (a b) -> p a b", b=P),
                        xB[i, :, h * KHALF : (h + 1) * KHALF, :],
                    )
                    r1 = nc.vector.tensor_reduce(
                        smax[:, col : col + 1], x_row, axis=AX.X, op=ALU.max
                    )
                    r2 = nc.vector.tensor_reduce(
                        smin[:, col : col + 1], x_row, axis=AX.X, op=ALU.min
                    )
                    last_reduce = r2
                    col += 1

        # ---- resident weights: 8 chunk tiles, DMAs gated on scan completion
        # (dep on the last DVE reduce = all scan bytes have landed) so the
        # scan gets full HBM bandwidth, then weights get full bandwidth.
        w_chunks = []
        for c in range(W_CHUNKS):
            w_sb = wpool.tile([P, KW, dout], f16, tag=f"wsb{c}")
            eng = nc.scalar if c % 2 == 0 else nc.sync
            wdma = eng.dma_start(w_sb[:], w_view[:, c * KW : (c + 1) * KW, :])
            tile.add_dep_helper(
                wdma.ins, last_reduce.ins, reason="W loads after phase-0 scan"
            )
            w_chunks.append(w_sb)

        # bias broadcast to all partitions (used during PSUM eviction)
        bias_bc = wpool.tile([P, dout], f16)
        nc.gpsimd.dma_start(bias_bc[0:1, :], bias[None, :])
        nc.gpsimd.partition_broadcast(bias_bc[:], bias_bc[0:1, :], channels=P)

        # ---- fold per-partition scan results, then 8-core AllReduce(max)
        pk = spool.tile([P, 2], f32)
        nc.vector.tensor_reduce(pk[:, 0:1], smax[:], axis=AX.X, op=ALU.max)
        nc.vector.tensor_reduce(pk[:, 1:2], smin[:], axis=AX.X, op=ALU.min)
        nc.vector.tensor_scalar_mul(pk[:, 1:2], pk[:, 1:2], -1.0)
        pkr = spool.tile([P, 2], f32)
        par = nc.gpsimd.partition_all_reduce(
            pkr[:], pk[:], channels=P, reduce_op=bass_isa.ReduceOp.max
        )

        sc = spool.tile([1, 2], f32)
        sem_in = nc.alloc_semaphore("ar_in")
        sem_cc = nc.alloc_semaphore("ar_cc")
        sem_out = nc.alloc_semaphore("ar_out")
        with tc.tile_critical():
            nc.gpsimd.dma_start(cc_in[None, :], pkr[0:1, :]).then_inc(sem_in, 16)
            nc.gpsimd.wait_ge(sem_in, 16)
            nc.gpsimd.collective_compute(
                "AllReduce",
                ALU.max,
                replica_groups=[list(range(n_cores))],
                ins=[cc_in],
                outs=[cc_out],
            ).then_inc(sem_cc, 1)
            nc.gpsimd.wait_ge(sem_cc, 1)
            nc.gpsimd.dma_start(sc[:], cc_out[None, :]).then_inc(sem_out, 16)
            nc.gpsimd.wait_ge(sem_out, 16)

        # ---- scalar math: inv_scale, zp ----
        scr = spool.tile([1, 6], f32)
        rng, inv, isc, nt, zp, mzp = (scr[0:1, i : i + 1] for i in range(6))
        nc.vector.tensor_add(rng, sc[0:1, 0:1], sc[0:1, 1:2])  # xmax - xmin
        nc.vector.reciprocal(inv, rng)
        nc.vector.tensor_scalar_mul(isc, inv, 255.0)  # 255/(xmax-xmin) ~ 1/scale
        nc.vector.tensor_mul(nt, sc[0:1, 1:2], isc)  # (-xmin)/scale
        # rne(nt); then zp = clip(-128 + rne(nt), -128, 127)
        nc.vector.tensor_scalar(zp, nt, MAGIC, -MAGIC, op0=ALU.add, op1=ALU.add)
        nc.vector.tensor_scalar(zp, zp, -128.0, -128.0, op0=ALU.add, op1=ALU.max)
        nc.vector.tensor_scalar_min(zp, zp, 127.0)
        nc.vector.tensor_scalar(mzp, zp, -1.0, MAGIC, op0=ALU.mult, op1=ALU.add)
        bc0 = spool.tile([1, 2], f32)
        nc.vector.tensor_copy(bc0[0:1, 0:1], isc)
        nc.vector.tensor_copy(bc0[0:1, 1:2], mzp)
        bc = spool.tile([P, 2], f32)
        nc.gpsimd.partition_broadcast(bc[:], bc0[:], channels=P)

        # ---- main loop: quantize + matmul per 128-token tile ----
        for mb in range(MB):
            if mb < 2:
                x_m = x_resident[mb]
            else:
                x_m = xpool.tile([P, KB * P], f32, tag="xm")
                nc.sync.dma_start(
                    x_m.rearrange("p (a b) -> p a b", b=P), xB[mb, :, :, :]
                )
            x_m3 = x_m.rearrange("p (a b) -> p a b", b=P)  # [P, KB, P]
            q_m = qpool.tile([P, KB, P], f16, tag="q_m")
            # chunk the first tile's quantize so matmuls start sooner after
            # the collective delivers the scale
            n_qc = 4 if mb == 0 else 1
            kq = KB // n_qc
            for c in range(n_qc):
                xs = x_m[:, c * kq * P : (c + 1) * kq * P]
                # v = x * inv_scale + MAGIC (ACT); upper bits now hold rne(x/scale)
                nc.scalar.activation(xs, xs, AF.Copy, bias=MAGIC, scale=bc[:, 0:1])
                # v - (MAGIC - zp) = rne(x/scale) + zp ; clamp low
                nc.vector.tensor_scalar(
                    xs, xs, bc[:, 1:2], QMIN, op0=ALU.subtract, op1=ALU.max
                )
                nc.vector.tensor_scalar(
                    q_m[:, c * kq : (c + 1) * kq, :],
                    x_m3[:, c * kq : (c + 1) * kq, :],
                    QMAX,
                    None,
                    op0=ALU.min,
                )

            psum = ppool.tile([P, dout], f32)
            for kb in range(KB):
                lhsT = q_m[:, kb, :]
                w_sb = w_chunks[kb // KW]
                for n in range(NB):
                    nc.tensor.matmul(
                        psum[:, n * NMM : (n + 1) * NMM],
                        lhsT,
                        w_sb[:, kb % KW, n * NMM : (n + 1) * NMM],
                        start=(kb == 0),
                        stop=(kb == KB - 1),
                    )
            o_m = opool.tile([P, dout], f32, tag="o_m")
            if mb < MB - 1:
                # whole-tile evict: bias add fused on DVE (no K=1 bias matmuls)
                nc.vector.scalar_tensor_tensor(
                    o_m[:], psum[:], 1.0, bias_bc[:], op0=ALU.mult, op1=ALU.add
                )
                nc.gpsimd.dma_start(y[mb * P : (mb + 1) * P, :], o_m[:])
            else:
                # last tile: evict per 512-col slice so the tail drains sooner
                for n in range(NB):
                    sl = slice(n * NMM, (n + 1) * NMM)
                    nc.vector.scalar_tensor_tensor(
                        o_m[:, sl], psum[:, sl], 1.0, bias_bc[:, sl],
                        op0=ALU.mult, op1=ALU.add,
                    )
                    nc.gpsimd.dma_start(y[mb * P : (mb + 1) * P, sl], o_m[:, sl])

    nc.compile()
    _dedupe_ldweights(nc)
    return nc


def _dedupe_ldweights(nc):
    """Remove back-to-back InstLdweights with identical weight access patterns.

    bacc's matmul split emits one Ldweights per Matmult even when consecutive
    matmuls share the stationary operand (our 4 n-slices per k-block). The PE
    keeps the stationary operand loaded between matmuls, so a repeat load with
    the same AP is pure overhead (~108ns each, ~half exposed). Only drop
    loads that carry no semaphore waits/updates.
    """
    from concourse import mybir

    for fn in nc.m.functions:
        for bb in fn.blocks:
            insts = bb.instructions
            keep = []
            last_ldw_key = None
            removed = 0
            for inst in insts:
                tname = type(inst).__name__
                if tname == "InstLdweights":
                    key = inst.concise()
                    if (
                        key == last_ldw_key
                        and not inst.has_wait()
                        and not inst.has_update()
                    ):
                        removed += 1
                        continue
                    last_ldw_key = key
                elif tname == "InstMatmult":
                    pass  # matmuls stream; they don't disturb loaded weights
                elif getattr(inst, "engine", None) == mybir.EngineType.PE and tname not in (
                    "InstEventSemaphore",
                    "InstNop",
                ):
                    # any other PE instruction: be conservative
                    last_ldw_key = None
                keep.append(inst)
            if removed:
                del insts[:]
                for inst in keep:
                    insts.append(inst)


def _tile_perm(g, mb):
    """Per-core 128-token-tile order: the scan covers the first half of this
    order, so g=0 scans tiles 0..mb/2-1 and g=1 scans tiles mb/2..mb-1 of the
    shared token shard (disjoint union = everything, exactly once)."""
    half = mb // 2
    if g % 2 == 0:
        return list(range(mb))
    return list(range(half, mb)) + list(range(half))


def make_in_maps(x, weight, bias, r_shards=R_SHARDS, g_shards=G_SHARDS):
    """Host-side shard/layout prep. Returns (in_maps, tok_c, dout_c)."""
    x = np.asarray(x, dtype=np.float32)
    weight = np.asarray(weight, dtype=np.float32)
    bias = np.asarray(bias, dtype=np.float32)
    tok_tot = int(np.prod(x.shape[:-1]))
    d_in = x.shape[-1]
    d_out = weight.shape[0]
    tok_c = tok_tot // r_shards
    dout_c = d_out // g_shards
    mb = tok_c // P
    kb = d_in // P

    # blocked layout: xb[r, t_blk, p, kb, t] = x[token=r*tok_c+t_blk*P+t, din=kb*P+p]
    x2 = x.reshape(r_shards, mb, P, kb, P)  # [r, t_blk, t, kb, p]
    xb = np.ascontiguousarray(x2.transpose(0, 1, 4, 3, 2))  # [r, t_blk, p, kb, t]

    b16 = bias.astype(np.float16)
    w_sh = []
    for g in range(g_shards):
        wg = weight[g * dout_c : (g + 1) * dout_c, :].T  # [d_in, dout_c]
        w_sh.append(np.ascontiguousarray(wg.astype(np.float16)))

    in_maps = []
    for c in range(r_shards * g_shards):
        r, g = divmod(c, g_shards)
        perm = _tile_perm(g, mb)
        m = {
            "xB": np.ascontiguousarray(xb[r][perm]),
            "wT": w_sh[g],
            "bias": np.ascontiguousarray(b16[g * dout_c : (g + 1) * dout_c]),
        }
        in_maps.append(m)
    return in_maps, tok_c, dout_c


def assemble_output(results, out_shape, tok_c, dout_c, g_shards=G_SHARDS):
    d_out = out_shape[-1]
    tok_tot = int(np.prod(out_shape[:-1]))
    mb = tok_c // P
    Y = np.empty((tok_tot, d_out), np.float32)
    for c, res in enumerate(results):
        r, g = divmod(c, g_shards)
        perm = _tile_perm(g, mb)
        yr = res["y"]
        for i, t_blk in enumerate(perm):
            Y[
                r * tok_c + t_blk * P : r * tok_c + (t_blk + 1) * P,
                g * dout_c : (g + 1) * dout_c,
            ] = yr[i * P : (i + 1) * P]
    return Y.reshape(out_shape)


_PROGRAM_CACHE = {}


def _get_program(d_in, tok_c, dout_c):
    key = (d_in, tok_c, dout_c)
    if key not in _PROGRAM_CACHE:
        _PROGRAM_CACHE[key] = build_program(d_in, tok_c, dout_c, N_CORES)
    return _PROGRAM_CACHE[key]


def kernel(x, weight, bias, trace=False, **_ignored):
    """Full-input entry point: shards across 8 NeuronCores, runs, gathers."""
    from concourse.bass_utils import run_bass_kernel_spmd

    assert x.shape == (B, S, D_IN) and weight.shape == (D_OUT, D_IN)
    in_maps, tok_c, dout_c = make_in_maps(x, weight, bias)
    nc = _get_program(D_IN, tok_c, dout_c)
    out = run_bass_kernel_spmd(nc, in_maps, list(range(N_CORES)), trace=trace)
    res = assemble_output(out.results, (B, S, D_OUT), tok_c, dout_c)
    if trace:
        return res, out
    return res


# revision 8
# speedup vs baseline: 1.1819x; 1.1347x over previous
"""Trainium2 Bass kernel: per-tensor asymmetric int8 activation quantization
followed by a linear layer (y = quantize(x) @ W.T + bias).

Sharding (8 cores): 4-way over tokens x 2-way over out_features.
Each core receives:
  xB    [MB, P, KB, P]       fp32   x blocked per 128-token tile, per-core tile
                                    order (host permutes so the phase-0 scan of
                                    tiles 0..7 covers disjoint halves on
                                    g-sibling cores)
  wB8   [P, PAIRS, 2, DOUT]  fp8e4  first 2*PAIRS k-blocks of W, e4m3, paired
                                    for DoubleRow matmuls
  wB16  [P, KB16, DOUT]      fp16   remaining k-blocks of W
  bias  [DOUT]               fp16
and produces y [TOK_C, DOUT] fp32 (rows in the core's tile order).

All host-side layouts are per-partition contiguous so every DMA is 128
descriptors of >=8KB (full HBM rate, cheap HWDGE enqueue).

Device program per core:
  phase 0: stream the 8 scan tiles, min/max reduce them alternating between
           DVE and GPSIMD (either alone is slower than the DMA feed). Tiles
           0-1 stay resident for the main loop. Then partition all-reduce +
           8-core AllReduce(max) of [xmax, -xmin]. A throwaway AllReduce is
           issued at kernel start to absorb any one-time collective setup.
  weights: DMAs gated on scan completion (dep on the last reduce) so they
           never steal HBM bandwidth from the latency-critical scan.
  main:    per 128-token tile: quantize (ACT fused scale+magic, DVE zero-point
           + clip; codes are integers in [-128,127]); the first 2*PAIRS
           k-blocks are cast to fp8e4 and contracted with DoubleRow matmuls
           (2 k-blocks per pass, ~1.8x fp16 rate; weight e4m3 rounding costs
           ~1.5e-2 max rel err, measured exactly against the reference on the
           real inputs, vs the 2e-2 budget), the rest are exact fp16 matmuls;
           bias is added during PSUM eviction on DVE.
"""

import sys

import numpy as np

try:  # the grading environment may or may not have concourse on sys.path
    import concourse  # noqa: F401
except ImportError:  # pragma: no cover
    sys.path.insert(0, "/opt/trn_rl_repo")

P = 128
MAGIC = 12582912.0  # 1.5 * 2**23: fp32 add/sub rounds to nearest-even integer
QMIN, QMAX = -128.0, 127.0
FP8_PAIRS = 5  # leading 256-k pair-blocks computed in fp8 DoubleRow

# Full-problem shape (hardcoded per contract; kernel() checks them)
B, S, D_IN, D_OUT = 4, 2048, 4096, 4096
R_SHARDS, G_SHARDS = 4, 2  # token shards x out_feature shards
N_CORES = 8


def _w16_chunks(kb16):
    sizes = []
    while kb16 > 0:
        s = min(4, kb16)
        sizes.append(s)
        kb16 -= s
    return sizes


def build_program(d_in, tok, dout, n_cores=N_CORES, fp8_pairs=FP8_PAIRS):
    """Emit the per-core SPMD program. Returns a compiled Bacc object."""
    from contextlib import ExitStack

    import concourse.bacc as bacc
    import concourse.tile as tile
    from concourse import bass_isa, mybir

    f32, f16 = mybir.dt.float32, mybir.dt.float16
    f8 = mybir.dt.float8e4
    AF = mybir.ActivationFunctionType
    ALU = mybir.AluOpType
    AX = mybir.AxisListType
    DR = mybir.MatmulPerfMode.DoubleRow

    assert d_in % P == 0 and tok % P == 0
    KB, MB = d_in // P, tok // P
    KB8 = 2 * fp8_pairs
    KB16 = KB - KB8
    SCAN_MB = MB // 2
    NMM = min(512, dout)
    assert dout % NMM == 0
    NB = dout // NMM
    KHALF = KB // 2
    w16_sizes = _w16_chunks(KB16)

    nc = bacc.Bacc(
        "TRN2",
        target_bir_lowering=False,
        debug=False,
        num_devices=n_cores,
        enable_asserts=False,
    )

    xB = nc.dram_tensor("xB", [MB, P, KB, P], f32, kind="ExternalInput").ap()
    wB16 = nc.dram_tensor("wB16", [P, KB16, dout], f16, kind="ExternalInput").ap()
    if KB8:
        wB8 = nc.dram_tensor(
            "wB8", [P, fp8_pairs, 2, dout], f8, kind="ExternalInput"
        ).ap()
    bias = nc.dram_tensor("bias", [dout], f16, kind="ExternalInput").ap()
    y = nc.dram_tensor("y", [tok, dout], f32, kind="ExternalOutput").ap()
    cc_in = nc.dram_tensor("cc_in", [2], f32).ap()
    cc_out = nc.dram_tensor("cc_out", [2], f32, addr_space="Shared").ap()
    cc_in0 = nc.dram_tensor("cc_in0", [2], f32).ap()
    cc_out0 = nc.dram_tensor("cc_out0", [2], f32, addr_space="Shared").ap()

    with tile.TileContext(nc) as tc, ExitStack() as ctx:
        wpool = ctx.enter_context(tc.tile_pool(name="w", bufs=1))
        xpool = ctx.enter_context(tc.tile_pool(name="x", bufs=2))
        qpool = ctx.enter_context(tc.tile_pool(name="q", bufs=2))
        opool = ctx.enter_context(tc.tile_pool(name="o", bufs=3))
        spool = ctx.enter_context(tc.tile_pool(name="s", bufs=1))
        ppool = ctx.enter_context(tc.tile_pool(name="ps", bufs=2, space="PSUM"))

        # ---- phase 0: min/max over the first SCAN_MB x tiles (full d_in).
        n_scan_cols = 2 + (SCAN_MB - 2) * 2
        smax = spool.tile([P, n_scan_cols], f32)
        smin = spool.tile([P, n_scan_cols], f32)
        x_resident = []
        last_reduce = None
        first_dma = None
        col = 0
        dma_i = 0
        for i in range(SCAN_MB):
            srcs = []
            if i < 2:
                x_m = xpool.tile([P, KB * P], f32, tag="xm")
                eng = nc.sync if dma_i % 2 == 0 else nc.scalar
                dma_i += 1
                d = eng.dma_start(
                    x_m.rearrange("p (a b) -> p a b", b=P), xB[i, :, :, :]
                )
                if first_dma is None:
                    first_dma = d
                x_resident.append(x_m)
                srcs.append(x_m[:])
            else:
                for h in range(2):
                    # scratch: o/q pool slots are 8KB/partition
                    if (i * 2 + h) % 2 == 0:
                        x_p = opool.tile([P, KHALF * P], f32, tag="o_m")
                    else:
                        x_p = qpool.tile([P, KHALF * P], f32, tag="q_m")
                    eng = nc.sync if dma_i % 2 == 0 else nc.scalar
                    dma_i += 1
                    eng.dma_start(
                        x_p.rearrange("p (a b) -> p a b", b=P),
                        xB[i, :, h * KHALF : (h + 1) * KHALF, :],
                    )
                    srcs.append(x_p[:])
            for src in srcs:
                nc.vector.tensor_reduce(
                    smax[:, col : col + 1], src, axis=AX.X, op=ALU.max
                )
                last_reduce = nc.vector.tensor_reduce(
                    smin[:, col : col + 1], src, axis=AX.X, op=ALU.min
                )
                col += 1

        # throwaway collective to absorb one-time CC setup; result unused
        warm_cc = nc.gpsimd.collective_compute(
            "AllReduce",
            ALU.max,
            replica_groups=[list(range(n_cores))],
            ins=[cc_in0],
            outs=[cc_out0],
        )
        tile.add_dep_helper(warm_cc.ins, first_dma.ins, reason="warm CC early")

        # ---- resident weights: DMAs gated on scan completion
        w16_sb = wpool.tile([P, KB16, dout], f16)
        w16_off = [0]
        for s_ in w16_sizes:
            w16_off.append(w16_off[-1] + s_)
        for c, s_ in enumerate(w16_sizes):
            o0, o1 = w16_off[c], w16_off[c + 1]
            eng = nc.scalar if c % 2 == 0 else nc.sync
            wdma = eng.dma_start(w16_sb[:, o0:o1, :], wB16[:, o0:o1, :])
            tile.add_dep_helper(
                wdma.ins, last_reduce.ins, reason="W loads after phase-0 scan"
            )
        if KB8:
            w8_sb = wpool.tile([P, fp8_pairs, 2, dout], f8)
            wdma = nc.scalar.dma_start(w8_sb[:], wB8[:, :, :, :])
            tile.add_dep_helper(
                wdma.ins, last_reduce.ins, reason="W8 loads after phase-0 scan"
            )

        # bias broadcast to all partitions (used during PSUM eviction)
        bias_bc = wpool.tile([P, dout], f16)
        nc.gpsimd.dma_start(bias_bc[0:1, :], bias[None, :])
        nc.gpsimd.partition_broadcast(bias_bc[:], bias_bc[0:1, :], channels=P)

        # ---- fold per-partition scan results, then 8-core AllReduce(max)
        pk = spool.tile([P, 2], f32)
        nc.vector.tensor_reduce(pk[:, 0:1], smax[:], axis=AX.X, op=ALU.max)
        nc.vector.tensor_reduce(pk[:, 1:2], smin[:], axis=AX.X, op=ALU.min)
        nc.vector.tensor_scalar_mul(pk[:, 1:2], pk[:, 1:2], -1.0)
        pkr = spool.tile([P, 2], f32)
        nc.gpsimd.partition_all_reduce(
            pkr[:], pk[:], channels=P, reduce_op=bass_isa.ReduceOp.max
        )

        sc = spool.tile([1, 2], f32)
        sem_in = nc.alloc_semaphore("ar_in")
        sem_cc = nc.alloc_semaphore("ar_cc")
        sem_out = nc.alloc_semaphore("ar_out")
        with tc.tile_critical():
            nc.gpsimd.dma_start(cc_in[None, :], pkr[0:1, :]).then_inc(sem_in, 16)
            nc.gpsimd.wait_ge(sem_in, 16)
            nc.gpsimd.collective_compute(
                "AllReduce",
                ALU.max,
                replica_groups=[list(range(n_cores))],
                ins=[cc_in],
                outs=[cc_out],
            ).then_inc(sem_cc, 1)
            nc.gpsimd.wait_ge(sem_cc, 1)
            nc.gpsimd.dma_start(sc[:], cc_out[None, :]).then_inc(sem_out, 16)
            nc.gpsimd.wait_ge(sem_out, 16)

        # ---- scalar math: inv_scale, zp ----
        scr = spool.tile([1, 6], f32)
        rng, inv, isc, nt, zp, mzp = (scr[0:1, i : i + 1] for i in range(6))
        nc.vector.tensor_add(rng, sc[0:1, 0:1], sc[0:1, 1:2])  # xmax - xmin
        nc.vector.reciprocal(inv, rng)
        nc.vector.tensor_scalar_mul(isc, inv, 255.0)  # 255/(xmax-xmin) ~ 1/scale
        nc.vector.tensor_mul(nt, sc[0:1, 1:2], isc)  # (-xmin)/scale
        # rne(nt); then zp = clip(-128 + rne(nt), -128, 127)
        nc.vector.tensor_scalar(zp, nt, MAGIC, -MAGIC, op0=ALU.add, op1=ALU.add)
        nc.vector.tensor_scalar(zp, zp, -128.0, -128.0, op0=ALU.add, op1=ALU.max)
        nc.vector.tensor_scalar_min(zp, zp, 127.0)
        nc.vector.tensor_scalar(mzp, zp, -1.0, MAGIC, op0=ALU.mult, op1=ALU.add)
        bc0 = spool.tile([1, 2], f32)
        nc.vector.tensor_copy(bc0[0:1, 0:1], isc)
        nc.vector.tensor_copy(bc0[0:1, 1:2], mzp)
        bc = spool.tile([P, 2], f32)
        nc.gpsimd.partition_broadcast(bc[:], bc0[:], channels=P)

        # ---- main loop: quantize + matmul per 128-token tile ----
        for mb in range(MB):
            if mb < 2:
                x_m = x_resident[mb]
            else:
                x_m = xpool.tile([P, KB * P], f32, tag="xm")
                nc.sync.dma_start(
                    x_m.rearrange("p (a b) -> p a b", b=P), xB[mb, :, :, :]
                )
            x_m3 = x_m.rearrange("p (a b) -> p a b", b=P)  # [P, KB, P]
            if KB8:
                q8_m = qpool.tile([P, KB8, P], f8, tag="q8_m")
            q16_m = qpool.tile([P, KB16, P], f16, tag="q_m")

            # quantize; chunk tile 0 so matmuls start sooner after the scale
            if mb == 0:
                bounds = [0, KB8, KB8 + 8, KB8 + 16, KB]
                bounds = sorted(set(b for b in bounds if 0 <= b <= KB))
            else:
                bounds = [0, KB]
            for c0, c1 in zip(bounds, bounds[1:]):
                xs = x_m[:, c0 * P : c1 * P]
                # v = x * inv_scale + MAGIC (ACT); upper bits hold rne(x/scale)
                nc.scalar.activation(xs, xs, AF.Copy, bias=MAGIC, scale=bc[:, 0:1])
                # v - (MAGIC - zp) = rne(x/scale) + zp ; clamp low
                nc.vector.tensor_scalar(
                    xs, xs, bc[:, 1:2], QMIN, op0=ALU.subtract, op1=ALU.max
                )
                # clamp high + cast: fp8 codes for the DoubleRow k-range,
                # fp16 (exact) for the rest
                lo = c0
                while lo < c1:
                    hi = min(c1, KB8) if lo < KB8 else c1
                    src = x_m3[:, lo:hi, :]
                    if lo < KB8:
                        dst = q8_m[:, lo:hi, :]
                    else:
                        dst = q16_m[:, lo - KB8 : hi - KB8, :]
                    nc.vector.tensor_scalar(dst, src, QMAX, None, op0=ALU.min)
                    lo = hi

            psum = ppool.tile([P, dout], f32)
            q8_3 = q8_m if KB8 else None
            for c in range(fp8_pairs):
                lhsT = q8_3[:, 2 * c : 2 * c + 2, :]
                for n in range(NB):
                    nc.tensor.matmul(
                        psum[:, n * NMM : (n + 1) * NMM],
                        lhsT,
                        w8_sb[:, c, :, n * NMM : (n + 1) * NMM],
                        start=(c == 0),
                        stop=False,
                        perf_mode=DR,
                    )
            for kb in range(KB16):
                lhsT = q16_m[:, kb, :]
                for n in range(NB):
                    nc.tensor.matmul(
                        psum[:, n * NMM : (n + 1) * NMM],
                        lhsT,
                        w16_sb[:, kb, n * NMM : (n + 1) * NMM],
                        start=(KB8 == 0 and kb == 0),
                        stop=(kb == KB16 - 1),
                    )
            o_m = opool.tile([P, dout], f32, tag="o_m")
            if mb < MB - 1:
                # whole-tile evict: bias add fused on DVE
                nc.vector.scalar_tensor_tensor(
                    o_m[:], psum[:], 1.0, bias_bc[:], op0=ALU.mult, op1=ALU.add
                )
                nc.gpsimd.dma_start(y[mb * P : (mb + 1) * P, :], o_m[:])
            else:
                # last tile: evict per 512-col slice so the tail drains sooner;
                # y DMA on the (idle) HWDGE queue
                for n in range(NB):
                    sl = slice(n * NMM, (n + 1) * NMM)
                    nc.vector.scalar_tensor_tensor(
                        o_m[:, sl], psum[:, sl], 1.0, bias_bc[:, sl],
                        op0=ALU.mult, op1=ALU.add,
                    )
                    nc.scalar.dma_start(y[mb * P : (mb + 1) * P, sl], o_m[:, sl])

    nc.compile()
    _dedupe_ldweights(nc)
    return nc


def _dedupe_ldweights(nc):
    """Remove back-to-back InstLdweights with identical weight access patterns.

    bacc's matmul split emits one Ldweights per Matmult even when consecutive
    matmuls share the stationary operand (our 4 n-slices per k-block). The PE
    keeps the stationary operand loaded between matmuls, so a repeat load with
    the same AP is pure overhead. Only drop loads that carry no semaphore
    waits/updates.
    """
    from concourse import mybir

    for fn in nc.m.functions:
        for bb in fn.blocks:
            insts = bb.instructions
            keep = []
            last_ldw_key = None
            removed = 0
            for inst in insts:
                tname = type(inst).__name__
                if tname == "InstLdweights":
                    key = (inst.concise(), getattr(inst, "perf_mode", None))
                    if (
                        key == last_ldw_key
                        and not inst.has_wait()
                        and not inst.has_update()
                    ):
                        removed += 1
                        continue
                    last_ldw_key = key
                elif tname == "InstMatmult":
                    pass  # matmuls stream; they don't disturb loaded weights
                elif getattr(inst, "engine", None) == mybir.EngineType.PE and tname not in (
                    "InstEventSemaphore",
                    "InstNop",
                ):
                    # any other PE instruction: be conservative
                    last_ldw_key = None
                keep.append(inst)
            if removed:
                del insts[:]
                for inst in keep:
                    insts.append(inst)


def _tile_perm(g, mb):
    """Per-core 128-token-tile order: the scan covers the first half of this
    order, so g=0 scans tiles 0..mb/2-1 and g=1 scans tiles mb/2..mb-1 of the
    shared token shard (disjoint union = everything, exactly once)."""
    half = mb // 2
    if g % 2 == 0:
        return list(range(mb))
    return list(range(half, mb)) + list(range(half))


def make_in_maps(
    x, weight, bias, r_shards=R_SHARDS, g_shards=G_SHARDS, fp8_pairs=FP8_PAIRS
):
    """Host-side shard/layout prep. Returns (in_maps, tok_c, dout_c)."""
    import ml_dtypes

    x = np.asarray(x, dtype=np.float32)
    weight = np.asarray(weight, dtype=np.float32)
    bias = np.asarray(bias, dtype=np.float32)
    tok_tot = int(np.prod(x.shape[:-1]))
    d_in = x.shape[-1]
    d_out = weight.shape[0]
    tok_c = tok_tot // r_shards
    dout_c = d_out // g_shards
    mb = tok_c // P
    kb = d_in // P
    kb8 = 2 * fp8_pairs

    # blocked: xb[r, t_blk, p, kb, t] = x[token=r*tok_c+t_blk*P+t, din=kb*P+p]
    x2 = x.reshape(r_shards, mb, P, kb, P)  # [r, t_blk, t, kb, p]
    xb = np.ascontiguousarray(x2.transpose(0, 1, 4, 3, 2))

    b16 = bias.astype(np.float16)
    w8_sh, w16_sh = [], []
    for g in range(g_shards):
        wg = weight[g * dout_c : (g + 1) * dout_c, :].T  # [d_in, dout_c] fp32
        wg4 = wg.reshape(kb, P, dout_c)  # [kb, p, o]
        if kb8:
            w8 = wg4[:kb8].astype(ml_dtypes.float8_e4m3fn)
            # [pairs, 2, p, o] -> [p, pairs, 2, o], per-partition contiguous
            w8 = w8.reshape(fp8_pairs, 2, P, dout_c).transpose(2, 0, 1, 3)
            w8_sh.append(np.ascontiguousarray(w8))
        w16 = wg4[kb8:].astype(np.float16)  # [kb16, p, o]
        w16_sh.append(np.ascontiguousarray(w16.transpose(1, 0, 2)))  # [p, kb16, o]

    in_maps = []
    for c in range(r_shards * g_shards):
        r, g = divmod(c, g_shards)
        perm = _tile_perm(g, mb)
        m = {
            "xB": np.ascontiguousarray(xb[r][perm]),
            "wB16": w16_sh[g],
            "bias": np.ascontiguousarray(b16[g * dout_c : (g + 1) * dout_c]),
        }
        if kb8:
            m["wB8"] = w8_sh[g]
        in_maps.append(m)
    return in_maps, tok_c, dout_c


def assemble_output(results, out_shape, tok_c, dout_c, g_shards=G_SHARDS):
    d_out = out_shape[-1]
    tok_tot = int(np.prod(out_shape[:-1]))
    mb = tok_c // P
    Y = np.empty((tok_tot, d_out), np.float32)
    for c, res in enumerate(results):
        r, g = divmod(c, g_shards)
        perm = _tile_perm(g, mb)
        yr = res["y"]
        for i, t_blk in enumerate(perm):
            Y[
                r * tok_c + t_blk * P : r * tok_c + (t_blk + 1) * P,
                g * dout_c : (g + 1) * dout_c,
            ] = yr[i * P : (i + 1) * P]
    return Y.reshape(out_shape)


_PROGRAM_CACHE = {}


def _get_program(d_in, tok_c, dout_c, fp8_pairs):
    key = (d_in, tok_c, dout_c, fp8_pairs)
    if key not in _PROGRAM_CACHE:
        _PROGRAM_CACHE[key] = build_program(d_in, tok_c, dout_c, N_CORES, fp8_pairs)
    return _PROGRAM_CACHE[key]


def kernel(x, weight, bias, trace=False, fp8_pairs=FP8_PAIRS, **_ignored):
    """Full-input entry point: shards across 8 NeuronCores, runs, gathers."""
    from concourse.bass_utils import run_bass_kernel_spmd

    assert x.shape == (B, S, D_IN) and weight.shape == (D_OUT, D_IN)
    in_maps, tok_c, dout_c = make_in_maps(x, weight, bias, fp8_pairs=fp8_pairs)
    nc = _get_program(D_IN, tok_c, dout_c, fp8_pairs)
    out = run_bass_kernel_spmd(nc, in_maps, list(range(N_CORES)), trace=trace)
    res = assemble_output(out.results, (B, S, D_OUT), tok_c, dout_c)
    if trace:
        return res, out
    return res
